# revision 1
# baseline (speedup 1.0000x reference)
"""Trainium2 Bass kernel for column-softmax attention.

reference semantics:
    scores = einsum('bqd,bkd->bqk', q, k) / sqrt(128)   # [B, Nq, Nk]
    attn   = softmax(scores, axis=1)                     # over the QUERY axis
    out    = einsum('bqk,bkd->bqd', attn, v)             # [B, Nq, D]

Because the softmax is over q, each key column k normalizes independently:
    out[q, d] = sum_k E[k, q] * r[k] * v[k, d],  E = exp(scores.T), r = 1/sum_q E[k, q]

Sharding: 8 cores = 4 batches x 2 key-halves.  Each core computes the partial
sum over its 2048 keys; the host adds the two partials per batch.

On-chip layout: the host pre-transposes Q and K to [D, N] (contraction dim on
partitions) and the kernel emits out.T [D, Nq]; the host transposes back.  The
softmax denominator is folded into V row-scaling so the normalize step touches
only 128x128 tiles per key tile.

Phase A (per key tile): scores matmul (fp16 in, fp32 psum) -> exp on ScalarE
(row-sums half fused into the activation, half on VectorE) -> E resident in
SBUF as fp16 -> this key tile's contribution to out.T for query half A
accumulated in PSUM (hides the second GEMM under the ScalarE exp span).
Phase B: query half B accumulated over all 16 key tiles, flushed, DMA'd out.

The ScalarE exp pass (8.4M elements/core, ~55us) is the roofline; measured
body time ~85-90us via the For_i loop-differencing method (see test.py).  PE
weight-load transitions cost ~1us each on this toolchain, so matmuls are
ordered to maximize consecutive same-stationary runs (explicit ordering deps
keep the scheduler from splitting them).
"""

import numpy as np

import concourse.bass as bass
import concourse.mybir as mybir
import concourse.tile as tile
from concourse.bass_utils import run_bass_kernel_spmd
from concourse.tile_rust import add_dep_helper

B, N, D = 4, 4096, 128
P = 128
NK = 2048                 # keys per core (half of 4096)
KT_TILES = NK // P        # 16 key tiles of 128
SCALE = 1.0 / np.sqrt(128.0)

F32 = mybir.dt.float32
F16 = mybir.dt.float16


def emit_body(nc, tc, pools, aps, skip_act=False, skip_phaseb=False, skip_gemm1=False, c2048=False, dve_rowsum=False):
    big, epool, small, spsum, opsum = pools
    qt_d, kt_d, v_d, out_d = aps

    qT = big.tile([P, N], F16, tag="qT")            # [d, q]
    kT = big.tile([P, NK], F16, tag="kT")           # [d, k]
    vsb = big.tile([P, KT_TILES, D], F16, tag="v")  # [k_in_tile, k_tile, d]
    oacc = big.tile([P, N], F32, tag="oacc")        # [d, q]

    for _qc in range(4):
        nc.sync.dma_start(
            qT[:, _qc * 1024 : (_qc + 1) * 1024], qt_d[:, _qc * 1024 : (_qc + 1) * 1024]
        )
    nc.sync.dma_start(kT[:], kt_d[:])
    nc.sync.dma_start(vsb[:], v_d.rearrange("(t p) d -> p t d", p=P))

    s_free = 2048 if c2048 else 1024
    # Warm-up matmul: first real matmul then carries at most one sync wait.
    Swarm = spsum.tile([P, s_free], F32, tag="S")
    nc.tensor.matmul(
        Swarm[0:1, 0:1], lhsT=kT[:, 0:1], rhs=qT[:, 0:1], start=True, stop=True
    )

    e_tiles = []
    v_tiles = []
    # Output accumulators for query half A (cols 0..2047) are built up during
    # phase A so most of the second GEMM hides under the exp (ScalarE) span.
    oa_tiles = []
    if not skip_phaseb and not c2048:
        for _oc in range(2):
            O_a = opsum.tile([P, 1024], F32, tag="O")
            oa_tiles.append(O_a)

    # Phase A: per key tile, scores + exp (row-sum fused) + scaled V,
    # then this key tile's contribution to out.T for query half A.
    for kt in range(KT_TILES):
        E = epool.tile([P, N], F16, tag=f"E{kt}")   # [k, q] = exp(scores.T)
        n_h = N // s_free
        rs = small.tile([P, n_h], F32, tag=f"rs{kt}")
        last_g1 = None
        for h in range(0 if skip_gemm1 else n_h):
            S = spsum.tile([P, s_free], F32, tag="S")
            for u in range(s_free // 512):
                last_g1 = nc.tensor.matmul(
                    S[:, u * 512 : (u + 1) * 512],
                    lhsT=kT[:, kt * P : (kt + 1) * P],
                    rhs=qT[:, h * s_free + u * 512 : h * s_free + u * 512 + 512],
                    start=True,
                    stop=True,
                )
            if not skip_act:
                if dve_rowsum and h < n_h // 2:
                    # row-sum for this chunk computed on VectorE from E
                    # (frees the ScalarE accumulator-read overhead)
                    nc.scalar.activation(
                        out=E[:, h * s_free : (h + 1) * s_free],
                        in_=S[:],
                        func=mybir.ActivationFunctionType.Exp,
                        scale=float(SCALE),
                    )
                    nc.vector.reduce_sum(
                        out=rs[:, h : h + 1],
                        in_=E[:, h * s_free : (h + 1) * s_free],
                        axis=mybir.AxisListType.X,
                    )
                else:
                    nc.scalar.activation(
                        out=E[:, h * s_free : (h + 1) * s_free],
                        in_=S[:],
                        func=mybir.ActivationFunctionType.Exp,
                        scale=float(SCALE),
                        accum_out=rs[:, h : h + 1],
                    )
        rsum = small.tile([P, 1], F32, tag="rsum")
        recip = small.tile([P, 1], F32, tag="recip")
        vsc = small.tile([P, D], F16, tag=f"vsc{kt}")  # [k, d] * r[k]
        if not skip_act:
            nc.vector.reduce_sum(out=rsum[:], in_=rs[:], axis=mybir.AxisListType.X)
            nc.vector.reciprocal(recip[:], rsum[:])
            nc.vector.tensor_scalar_mul(vsc[:], vsb[:, kt, :], recip[:])
        elif not skip_phaseb:
            nc.sync.dma_start(E[:], qt_d[:])
            nc.vector.tensor_copy(out=vsc[:], in_=vsb[:, kt, :])
        e_tiles.append(E)
        v_tiles.append(vsc)

        if not skip_phaseb and not c2048:
            # Emit the PREVIOUS key tile's half-A output matmuls here, ordered
            # after this tile's scores matmuls (ordering-only deps).  This
            # keeps each kT weight-load run contiguous: the scheduler would
            # otherwise wedge the vsc burst into the middle of the scores
            # run, costing an extra ~1us weight transition per key tile.
            if kt > 0:
                pv, pe_t, pkt = pending_g2a
                for oc in range(2):
                    for u in range(2):
                        mm = nc.tensor.matmul(
                            oa_tiles[oc][:, u * 512 : (u + 1) * 512],
                            lhsT=pv[:],
                            rhs=pe_t[:, oc * 1024 + u * 512 : oc * 1024 + (u + 1) * 512],
                            start=(pkt == 0),
                            stop=False,
                        )
                        if last_g1 is not None:
                            add_dep_helper(
                                mm.ins,
                                last_g1.ins,
                                sync=False,
                                reason="keep kT weight-load run contiguous",
                            )
            pending_g2a = (vsc, E, kt)

    if not skip_phaseb and not c2048:
        pv, pe_t, pkt = pending_g2a
        for oc in range(2):
            for u in range(2):
                nc.tensor.matmul(
                    oa_tiles[oc][:, u * 512 : (u + 1) * 512],
                    lhsT=pv[:],
                    rhs=pe_t[:, oc * 1024 + u * 512 : oc * 1024 + (u + 1) * 512],
                    start=False,
                    stop=True,
                )

    # Phase B: flush half A, then accumulate query half B (cols 2048..4095).
    if not skip_phaseb and c2048:
        # no interleave: both halves accumulated here, kt-outer.
        # O tiles live in the S pool slots (the opsum pool has no banks
        # left when S is [128, 2048] double-buffered).
        for half in range(2):
            hb_tiles = []
            for _oc in range(2):
                O_h = spsum.tile([P, 1024], F32, tag="S")
                hb_tiles.append(O_h)
            for kt in range(KT_TILES):
                for oc in range(2):
                    for u in range(2):
                        base = half * 2048 + oc * 1024 + u * 512
                        nc.tensor.matmul(
                            hb_tiles[oc][:, u * 512 : (u + 1) * 512],
                            lhsT=v_tiles[kt][:],
                            rhs=e_tiles[kt][:, base : base + 512],
                            start=(kt == 0),
                            stop=(kt == KT_TILES - 1),
                        )
            for oc in range(2):
                lo = half * 2048 + oc * 1024
                nc.vector.tensor_copy(out=oacc[:, lo : lo + 1024], in_=hb_tiles[oc][:])
                nc.sync.dma_start(out_d[:, lo : lo + 1024], oacc[:, lo : lo + 1024])
    elif not skip_phaseb:
        for oc in range(2):
            nc.vector.tensor_copy(
                out=oacc[:, oc * 1024 : (oc + 1) * 1024], in_=oa_tiles[oc][:]
            )
            nc.sync.dma_start(
                out_d[:, oc * 1024 : (oc + 1) * 1024],
                oacc[:, oc * 1024 : (oc + 1) * 1024],
            )
        ob_tiles = []
        for _oc in range(2):
            O_b = opsum.tile([P, 1024], F32, tag="O")
            ob_tiles.append(O_b)
        prev_mm = None
        for kt in range(KT_TILES):
            for oc in range(2):
                for u in range(2):
                    mm = nc.tensor.matmul(
                        ob_tiles[oc][:, u * 512 : (u + 1) * 512],
                        lhsT=v_tiles[kt][:],
                        rhs=e_tiles[kt][:, 2048 + oc * 1024 + u * 512 : 2048 + oc * 1024 + (u + 1) * 512],
                        start=(kt == 0),
                        stop=(kt == KT_TILES - 1),
                    )
                    # chain ordering so each vsc weight-load run stays a
                    # contiguous block of 4 (scheduler otherwise splits the
                    # first few key tiles into 2+2 across the O tiles)
                    if prev_mm is not None:
                        add_dep_helper(
                            mm.ins, prev_mm.ins, sync=False,
                            reason="contiguous vsc weight runs in tail",
                        )
                    prev_mm = mm
        for oc in range(2):
            nc.vector.tensor_copy(
                out=oacc[:, 2048 + oc * 1024 : 2048 + (oc + 1) * 1024],
                in_=ob_tiles[oc][:],
            )
            nc.sync.dma_start(
                out_d[:, 2048 + oc * 1024 : 2048 + (oc + 1) * 1024],
                oacc[:, 2048 + oc * 1024 : 2048 + (oc + 1) * 1024],
            )
    else:
        nc.gpsimd.memset(oacc[:], 0.0)
        nc.sync.dma_start(out_d[:], oacc[:])


def build_bass(repeat=1, skip_act=False, skip_phaseb=False, skip_gemm1=False, loop=False, c2048=False, dve_rowsum=False):
    nc = bass.Bass("TRN2", target_bir_lowering=False, debug=False)
    qt_d = nc.dram_tensor("qt", [P, N], F16, kind="ExternalInput").ap()
    kt_d = nc.dram_tensor("kt", [P, NK], F16, kind="ExternalInput").ap()
    v_d = nc.dram_tensor("v", [NK, D], F16, kind="ExternalInput").ap()
    out_d = nc.dram_tensor("out_t", [P, N], F32, kind="ExternalOutput").ap()

    with tile.TileContext(nc) as tc:
        import contextlib
        with (
            tc.tile_pool(name="big", bufs=1) as big,
            tc.tile_pool(name="epool", bufs=1) as epool,
            tc.tile_pool(name="small", bufs=2) as small,
            tc.tile_pool(name="spsum", bufs=2, space="PSUM") as spsum,
            (contextlib.nullcontext(None) if c2048
             else tc.tile_pool(name="opsum", bufs=2, space="PSUM")) as opsum,
        ):
            def body():
                emit_body(
                    nc,
                    tc,
                    (big, epool, small, spsum, opsum),
                    (qt_d, kt_d, v_d, out_d),
                    skip_act=skip_act,
                    skip_phaseb=skip_phaseb,
                    skip_gemm1=skip_gemm1,
                    c2048=c2048,
                    dve_rowsum=dve_rowsum,
                )

            if loop and repeat > 1:
                with tc.For_i(
                    0, repeat, 1,
                    hint_engines=(mybir.EngineType.PE, mybir.EngineType.Activation),
                ):
                    body()
            else:
                for _ in range(repeat):
                    body()
    return nc


def legalize_waits(nc, max_waits=1):
    """Hoist excess semaphore waits into standalone EventSemaphore ops.

    The walrus codegen for several engine instruction structs accepts only a
    single sync-wait command; Tile sometimes emits more.  Executing the extra
    waits in a preceding same-engine EventSemaphore is semantically identical
    (the engine runs its stream in order).
    """
    for fn in nc.m.functions:
        for blk in fn.blocks:
            out = []
            for inst in blk.instructions:
                si = inst.sync_info
                if (
                    si is not None
                    and si.on_wait
                    and len(si.on_wait) > max_waits
                    and inst.opcode != "EventSemaphore"
                ):
                    waits = list(si.on_wait)
                    extra, keep = waits[:-max_waits], waits[-max_waits:]
                    for n, w in enumerate(extra):
                        out.append(
                            mybir.InstEventSemaphore(
                                name=f"{inst.name}_prewait{n}",
                                engine=inst.engine,
                                ins=[],
                                outs=[],
                                sync_info=mybir.SyncInfo(on_wait=[w], on_update=[]),
                            )
                        )
                    si.on_wait = keep
                out.append(inst)
            blk.instructions = out
    return nc


_NC_CACHE = {}


def _get_nc(repeat=1, **kw):
    key = ("nc", repeat, tuple(sorted(kw.items())))
    if key not in _NC_CACHE:
        _NC_CACHE[key] = legalize_waits(build_bass(repeat, **kw))
    return _NC_CACHE[key]


def kernel(q, k, v):
    q = np.asarray(q, dtype=np.float32)
    k = np.asarray(k, dtype=np.float32)
    v = np.asarray(v, dtype=np.float32)

    in_maps = []
    for c in range(8):
        b, h = c // 2, c % 2
        in_maps.append(
            {
                "qt": np.ascontiguousarray(q[b].T).astype(np.float16),
                "kt": np.ascontiguousarray(k[b, h * NK : (h + 1) * NK].T).astype(np.float16),
                "v": np.ascontiguousarray(v[b, h * NK : (h + 1) * NK]).astype(np.float16),
            }
        )

    nc = _get_nc()
    res = run_bass_kernel_spmd(nc, in_maps, list(range(8))).results

    out = np.empty((B, N, D), dtype=np.float32)
    for b in range(B):
        out[b] = (res[2 * b]["out_t"] + res[2 * b + 1]["out_t"]).T
    return out



# revision 11
# speedup vs baseline: 1.3425x; 1.3425x over previous
"""Trainium2 Bass kernel for column-softmax attention.

reference semantics:
    scores = einsum('bqd,bkd->bqk', q, k) / sqrt(128)   # [B, Nq, Nk]
    attn   = softmax(scores, axis=1)                     # over the QUERY axis
    out    = einsum('bqk,bkd->bqd', attn, v)             # [B, Nq, D]

Because the softmax is over q, each key column k normalizes independently:
    out[q, d] = sum_k E[k, q] * r[k] * v[k, d],  E = exp(scores.T), r = 1/sum_q E[k, q]

Sharding: 8 cores = 4 batches x 2 key-halves.  Each core computes the partial
sum over its 2048 keys; the host adds the two partials per batch.

On-chip layout: the host pre-transposes Q and K to [D, N] (contraction dim on
partitions) and the kernel emits out.T [D, Nq]; the host transposes back.  The
softmax denominator is folded into V row-scaling so the normalize step touches
only 128x128 tiles per key tile.

The ScalarE exp pass (8.4M elements/core) is the roofline: 64 ACTIVATE
instructions of N=1024 from PSUM = ~66us engine-busy.  v2 structure:
  - row sums moved OFF ScalarE (no accum_out, which costs an extra ~190-280ns
    per ACTIVATE): one DVE reduce per key tile over the fp16 E row (2x mode).
  - startup: first key tile + first q chunk DMA'd first; a dummy exp on a
    memset tile preloads the ACT table under the DMA window.
  - phase A (per key tile): scores matmul -> exp -> E resident fp16; previous
    tile's out.T contribution for query half A accumulated in PSUM (hides the
    second GEMM under the ScalarE exp span).
  - tail: query half B accumulated in the freed S-pool PSUM banks (so it does
    not wait on the half-A flush), flush copies split ACT/DVE, DMAs overlap.
PE weight-load transitions are kept contiguous per stationary via explicit
ordering deps (the scheduler would otherwise split runs, ~1us per transition).
"""

import numpy as np

import concourse.bass as bass
import concourse.mybir as mybir
import concourse.tile as tile
from concourse.bass_utils import run_bass_kernel_spmd
from concourse.tile_rust import add_dep_helper

B, N, D = 4, 4096, 128
P = 128
NK = 2048                 # keys per core (half of 4096)
KT_TILES = NK // P        # 16 key tiles of 128
SCALE = 1.0 / np.sqrt(128.0)

F32 = mybir.dt.float32
F16 = mybir.dt.float16


def emit_body(nc, tc, pools, aps):
    big, epool, small, spsum, opsum = pools
    qt_d, kt_d, v_d, out_d = aps

    qT = big.tile([P, N], F16, tag="qT")            # [d, q]
    kT = big.tile([P, NK], F16, tag="kT")           # [d, k]
    vsb = big.tile([P, KT_TILES, D], F16, tag="v")  # [k_in_tile, k_tile, d]
    oacc = big.tile([P, N], F16, tag="oacc")        # [d, q] fp16 staging

    # DMA order: the first scores matmul needs kT tile 0 + qT[0:512] only.
    nc.sync.dma_start(qT[:, 0:512], qt_d[:, 0:512])
    nc.sync.dma_start(kT[:, 0:P], kt_d[:, 0:P])
    nc.sync.dma_start(qT[:, 512:1024], qt_d[:, 512:1024])
    for _qc in range(1, 4):
        nc.sync.dma_start(
            qT[:, _qc * 1024 : (_qc + 1) * 1024], qt_d[:, _qc * 1024 : (_qc + 1) * 1024]
        )
    # kT tiles 1.. are first needed at tile-1 scores (~7us in), q chunks first.
    nc.sync.dma_start(kT[:, P:NK], kt_d[:, P:NK])
    nc.sync.dma_start(vsb[:], v_d.rearrange("(t p) d -> p t d", p=P))

    # ACT table preload: dummy exp on a memset tile runs under the DMA window,
    # so the ~1.3us exp_and_others table load is off the critical path.
    warm_in = small.tile([P, 1], F32, tag="warm_in")
    warm_out = small.tile([P, 1], F32, tag="warm_out")
    nc.gpsimd.memset(warm_in[:], 0.0)
    nc.scalar.activation(
        out=warm_out[:], in_=warm_in[:], func=mybir.ActivationFunctionType.Exp
    )

    # Warm-up matmul: first real matmul then carries at most one sync wait.
    Swarm = spsum.tile([P, 1024], F32, tag="S")
    nc.tensor.matmul(
        Swarm[0:1, 0:1], lhsT=kT[:, 0:1], rhs=qT[:, 0:1], start=True, stop=True
    )

    e_tiles = []
    v_tiles = []
    # Output accumulators for query half A (cols 0..2047) are built up during
    # phase A so the second GEMM's first half hides under the exp span.
    oa_tiles = []
    for _oc in range(2):
        O_a = opsum.tile([P, 1024], F32, tag="O")
        oa_tiles.append(O_a)

    pending_g2a = None
    for kt in range(KT_TILES):
        last_tile = kt == KT_TILES - 1
        E = epool.tile([P, N], F16, tag=f"E{kt}")   # [k, q] = exp(scores.T)
        rs = small.tile([P, 4], F32, tag="rs")
        last_g1 = None
        for h in range(4):
            S = spsum.tile([P, 1024], F32, tag="S")
            for u in range(2):
                last_g1 = nc.tensor.matmul(
                    S[:, u * 512 : (u + 1) * 512],
                    lhsT=kT[:, kt * P : (kt + 1) * P],
                    rhs=qT[:, h * 1024 + u * 512 : h * 1024 + u * 512 + 512],
                    start=True,
                    stop=True,
                )
            # For the LAST tile only, fuse the row sums into the activation
            # (accum_out): the DVE reduce chain would otherwise put ~3us of
            # vsc15 latency on the critical path into the tail.  For all
            # other tiles the fused accumulator read costs ~190-280ns of
            # ScalarE per chunk, so the row sums go to DVE instead.
            nc.scalar.activation(
                out=E[:, h * 1024 : (h + 1) * 1024],
                in_=S[:],
                func=mybir.ActivationFunctionType.Exp,
                scale=float(SCALE),
                accum_out=rs[:, h : h + 1] if last_tile else None,
            )
        rsum = small.tile([P, 1], F32, tag="rsum")
        recip = small.tile([P, 1], F32, tag="recip")
        vsc = small.tile([P, D], F16, tag=f"vsc{kt}")  # [k, d] * r[k]
        if last_tile:
            nc.vector.reduce_sum(out=rsum[:], in_=rs[:], axis=mybir.AxisListType.X)
        else:
            # Row sums on DVE, off the ScalarE path.  A straight TensorReduce
            # over [128, 4096] runs at 1 elem/cycle = ~4.3us/tile and would
            # make DVE the per-tile bottleneck; fp16 pairwise adds get the
            # 2x DVE rate, so fold 4096 -> 512 in three adds + short reduce
            # (~2.9us).  (tensor_tensor_reduce would be one instruction but
            # does not compile on this toolchain: "ISA wrong length".)
            t1 = small.tile([P, 2048], F16, tag="rt1")
            t2 = small.tile([P, 1024], F16, tag="rt2")
            t3 = small.tile([P, 512], F16, tag="rt3")
            nc.vector.tensor_add(t1[:], E[:, 0:2048], E[:, 2048:4096])
            nc.vector.tensor_add(t2[:], t1[:, 0:1024], t1[:, 1024:2048])
            nc.vector.tensor_add(t3[:], t2[:, 0:512], t2[:, 512:1024])
            nc.vector.reduce_sum(out=rsum[:], in_=t3[:], axis=mybir.AxisListType.X)
        nc.vector.reciprocal(recip[:], rsum[:])
        nc.vector.tensor_scalar_mul(vsc[:], vsb[:, kt, :], recip[:])
        e_tiles.append(E)
        v_tiles.append(vsc)

        # Emit the PREVIOUS key tile's half-A output matmuls here, ordered
        # after this tile's scores matmuls (ordering-only deps).  This keeps
        # each kT weight-load run contiguous.
        if kt > 0:
            pv, pe_t, pkt = pending_g2a
            for oc in range(2):
                for u in range(2):
                    mm = nc.tensor.matmul(
                        oa_tiles[oc][:, u * 512 : (u + 1) * 512],
                        lhsT=pv[:],
                        rhs=pe_t[:, oc * 1024 + u * 512 : oc * 1024 + (u + 1) * 512],
                        start=(pkt == 0),
                        stop=False,
                    )
                    if last_g1 is not None:
                        add_dep_helper(
                            mm.ins,
                            last_g1.ins,
                            sync=False,
                            reason="keep kT weight-load run contiguous",
                        )
        pending_g2a = (vsc, E, kt)

    # Close the half-A accumulation with the last key tile's contribution.
    pv, pe_t, pkt = pending_g2a
    last_close = None
    for oc in range(2):
        for u in range(2):
            last_close = nc.tensor.matmul(
                oa_tiles[oc][:, u * 512 : (u + 1) * 512],
                lhsT=pv[:],
                rhs=pe_t[:, oc * 1024 + u * 512 : oc * 1024 + (u + 1) * 512],
                start=False,
                stop=True,
            )

    # Tail: query half B accumulates in the S-pool banks (free once the last
    # exp has read them) so it does NOT wait on the half-A flush below.
    ob_tiles = []
    for _oc in range(2):
        O_b = spsum.tile([P, 1024], F32, tag="S")
        ob_tiles.append(O_b)
    prev_mm = None
    for kt in range(KT_TILES):
        for oc in range(2):
            for u in range(2):
                mm = nc.tensor.matmul(
                    ob_tiles[oc][:, u * 512 : (u + 1) * 512],
                    lhsT=v_tiles[kt][:],
                    rhs=e_tiles[kt][:, 2048 + oc * 1024 + u * 512 : 2048 + oc * 1024 + (u + 1) * 512],
                    start=(kt == 0),
                    stop=(kt == KT_TILES - 1),
                )
                # chain ordering so each vsc weight-load run stays a
                # contiguous block of 4, and so the half-A close runs FIRST
                # (otherwise the scheduler defers it to the very end and the
                # half-A flush + DMAs pile up behind the half-B tail)
                add_dep_helper(
                    mm.ins,
                    (prev_mm or last_close).ins,
                    sync=False,
                    reason="contiguous vsc weight runs in tail",
                )
                prev_mm = mm

    # Flushes: fp16 staging (halves the out DMA bytes; the host adds the two
    # per-batch partials in fp32).  Copies split across ACT (idle in the
    # tail) and DVE in 512-col quarters so the last copy->DMA chain is short;
    # each quarter's DMA issues as soon as it is staged.
    def flush(o_pair, lo):
        for oc in range(2):
            for qr in range(2):
                src = o_pair[oc][:, qr * 512 : (qr + 1) * 512]
                dst = oacc[:, lo + oc * 1024 + qr * 512 : lo + oc * 1024 + (qr + 1) * 512]
                if oc == 0:
                    nc.scalar.copy(out=dst, in_=src)
                else:
                    nc.vector.tensor_copy(out=dst, in_=src)
                nc.sync.dma_start(
                    out_d[:, lo + oc * 1024 + qr * 512 : lo + oc * 1024 + (qr + 1) * 512],
                    dst,
                )

    flush(oa_tiles, 0)
    flush(ob_tiles, 2048)


def build_bass(repeat=1, loop=False):
    nc = bass.Bass("TRN2", target_bir_lowering=False, debug=False)
    qt_d = nc.dram_tensor("qt", [P, N], F16, kind="ExternalInput").ap()
    kt_d = nc.dram_tensor("kt", [P, NK], F16, kind="ExternalInput").ap()
    v_d = nc.dram_tensor("v", [NK, D], F16, kind="ExternalInput").ap()
    out_d = nc.dram_tensor("out_t", [P, N], F16, kind="ExternalOutput").ap()

    with tile.TileContext(nc) as tc:
        with (
            tc.tile_pool(name="big", bufs=1) as big,
            tc.tile_pool(name="epool", bufs=1) as epool,
            tc.tile_pool(name="small", bufs=2) as small,
            tc.tile_pool(name="spsum", bufs=2, space="PSUM") as spsum,
            tc.tile_pool(name="opsum", bufs=2, space="PSUM") as opsum,
        ):
            def body():
                emit_body(
                    nc,
                    tc,
                    (big, epool, small, spsum, opsum),
                    (qt_d, kt_d, v_d, out_d),
                )

            if loop and repeat > 1:
                with tc.For_i(
                    0, repeat, 1,
                    hint_engines=(mybir.EngineType.PE, mybir.EngineType.Activation),
                ):
                    body()
            else:
                for _ in range(repeat):
                    body()
    return nc


def legalize_waits(nc, max_waits=1):
    """Hoist excess semaphore waits into standalone EventSemaphore ops.

    The walrus codegen for several engine instruction structs accepts only a
    single sync-wait command; Tile sometimes emits more.  Executing the extra
    waits in a preceding same-engine EventSemaphore is semantically identical
    (the engine runs its stream in order).
    """
    for fn in nc.m.functions:
        for blk in fn.blocks:
            out = []
            for inst in blk.instructions:
                si = inst.sync_info
                if (
                    si is not None
                    and si.on_wait
                    and len(si.on_wait) > max_waits
                    and inst.opcode != "EventSemaphore"
                ):
                    waits = list(si.on_wait)
                    extra, keep = waits[:-max_waits], waits[-max_waits:]
                    for n, w in enumerate(extra):
                        out.append(
                            mybir.InstEventSemaphore(
                                name=f"{inst.name}_prewait{n}",
                                engine=inst.engine,
                                ins=[],
                                outs=[],
                                sync_info=mybir.SyncInfo(on_wait=[w], on_update=[]),
                            )
                        )
                    si.on_wait = keep
                out.append(inst)
            blk.instructions = out
    return nc


_NC_CACHE = {}


def _get_nc(repeat=1, **kw):
    key = ("nc", repeat, tuple(sorted(kw.items())))
    if key not in _NC_CACHE:
        _NC_CACHE[key] = legalize_waits(build_bass(repeat, **kw))
    return _NC_CACHE[key]


def kernel(q, k, v):
    q = np.asarray(q, dtype=np.float32)
    k = np.asarray(k, dtype=np.float32)
    v = np.asarray(v, dtype=np.float32)

    in_maps = []
    for c in range(8):
        b, h = c // 2, c % 2
        in_maps.append(
            {
                "qt": np.ascontiguousarray(q[b].T).astype(np.float16),
                "kt": np.ascontiguousarray(k[b, h * NK : (h + 1) * NK].T).astype(np.float16),
                "v": np.ascontiguousarray(v[b, h * NK : (h + 1) * NK]).astype(np.float16),
            }
        )

    nc = _get_nc()
    res = run_bass_kernel_spmd(nc, in_maps, list(range(8))).results

    out = np.empty((B, N, D), dtype=np.float32)
    for b in range(B):
        out[b] = (
            res[2 * b]["out_t"].astype(np.float32)
            + res[2 * b + 1]["out_t"].astype(np.float32)
        ).T
    return out
